# revision 3
# baseline (speedup 1.0000x reference)
"""Trainium2 Bass kernel for IterTranslatedSpatialCorrelationSampler.

Math: for each pixel P=(y,x), base = P + flow(P); x0=floor(base_x), y0=floor(base_y),
fx,fy fractional. All 81 patch offsets share (fx,fy), so
  corr[dy,dx] = (1-fy)*Fy[dy+4, .] + ... factors through the integer-displacement
  volume V(y0+j-4, x0+i-4) = sum_c i1[c,P] * i2[c, r, s].
We fold the y-blend into the matmul via two host-prescaled weight sets
  w0 = (1-fy)*i1, w1 = fy*i1, giving per pixel
  G[a, s] = sum_c w0[c]*i2[c, y0+a-4, s] + w1[c]*i2[c, y0+a-4+1, s],  a=0..8
over a 25-wide column window (pixels bucketed by (y0, x-band of 16)).
Then on-chip: barrel-shift extract the 10-wide per-pixel window (off = x0 mod 16
via 4 predicated-select stages), and the final x-blend
  corr[a, dx] = G[a, off+dx] + fx*(G[a, off+dx+1] - G[a, off+dx]).

Sharding: data-parallel over batch, core b <- batch item b. i2 (zero-padded,
bf16) is resident in SBUF; w0/w1 pixel columns are gathered per 128-slot pass
with dma_gather(transpose=True) directly into matmul lhsT layout. 4 pixel
chunks (<=32 each, same (y0,xband)) are packed per 128-partition pass with
tile_position col-tiling. Output is written slot-major and unpermuted on host.
"""

import math
import numpy as np
import ml_dtypes

import concourse.bass as bass
import concourse.mybir as mybir
import concourse.tile as tile
from concourse import bacc
from concourse import bass_utils

B, C, H, W = 8, 256, 64, 128
HW = H * W
U = 10          # integer window rows per pixel
TW = 10         # integer window cols per pixel
XBW = 16        # x-band width (pixels bucketed by x0 // XBW)
WIN = XBW + TW - 1   # 25: rhs window width per chunk
MSUB = 32       # pixels per col-tile chunk
NB = 8          # passes per post-processing batch

_bf16 = ml_dtypes.bfloat16


def _host_prep(input1, input2, flow):
    """All data-dependent structure, computed once on host (numpy)."""
    ys, xs = np.meshgrid(np.arange(H, dtype=np.float32),
                         np.arange(W, dtype=np.float32), indexing="ij")
    bx = xs[None] + flow[:, 0]           # (B,H,W)
    by = ys[None] + flow[:, 1]
    x0 = np.floor(bx).astype(np.int64)
    y0 = np.floor(by).astype(np.int64)
    fx = (bx - x0).astype(np.float32)
    fy = (by - y0).astype(np.float32)

    # window rows [y0-4, y0+5] must intersect [0, H); cols [x0-4, x0+5] must
    # intersect [0, W)
    part = (y0 >= -5) & (y0 <= H + 3) & (x0 >= -5) & (x0 <= W + 3)

    xb = np.floor_divide(x0, XBW)

    # --- shared chunk grid across all cores ---
    buckets = [dict() for _ in range(B)]   # key -> list of pixel ids
    for b in range(B):
        yy = y0[b].ravel(); xx = xb[b].ravel(); pp = part[b].ravel()
        ids = np.nonzero(pp)[0]
        order = np.lexsort((xx[ids], yy[ids]))
        ids = ids[order]
        keys = list(zip(yy[ids].tolist(), xx[ids].tolist()))
        d = buckets[b]
        prev = None; cur = None
        for pid, k in zip(ids.tolist(), keys):
            if k != prev:
                cur = d.setdefault(k, [])
                prev = k
            cur.append(pid)

    all_keys = sorted(set().union(*[set(d.keys()) for d in buckets]))
    chunk_list = []          # (y0, xb) per chunk
    chunk_of_key = {}        # key -> (first chunk index, nchunks)
    for k in all_keys:
        cap = max(len(d.get(k, ())) for d in buckets)
        nch = math.ceil(cap / MSUB)
        chunk_of_key[k] = (len(chunk_list), nch)
        chunk_list += [k] * nch
    while len(chunk_list) % 4:
        chunk_list.append(chunk_list[-1])
        # note: duplicated chunk gets no pixels (cap already covered)
    NP = len(chunk_list) // 4            # number of passes

    # --- per-core slot tables ---
    NSLOT = 128 * NP
    idx = np.zeros((B, NSLOT), np.int64)          # slot -> pixel id (0 if pad)
    pixmap = np.full((B, NSLOT), -1, np.int64)    # slot -> pixel id (-1 if pad)
    filled = np.zeros((B, NSLOT), bool)
    for b in range(B):
        d = buckets[b]
        for k, pids in d.items():
            c0, nch = chunk_of_key[k]
            for j, pid in enumerate(pids):
                ch = c0 + j // MSUB
                lane = j % MSUB
                s = (ch // 4) * 128 + (ch % 4) * MSUB + lane
                idx[b, s] = pid
                pixmap[b, s] = pid
                filled[b, s] = True

    fxr = fx.reshape(B, HW); fyr = fy.reshape(B, HW)
    x0r = x0.reshape(B, HW); xbr = xb.reshape(B, HW)

    fx_slot = np.where(filled, fxr[np.arange(B)[:, None], idx], 0.0).astype(np.float32)
    off_slot = np.where(filled,
                        x0r[np.arange(B)[:, None], idx] - XBW * xbr[np.arange(B)[:, None], idx],
                        0).astype(np.int64)
    assert off_slot.min() >= 0 and off_slot.max() < XBW

    # reshape slot tables to [128, NP] device layout: slot s -> (lane s%128, pass s//128)
    def slot2dev(a, dt):
        return np.ascontiguousarray(a.reshape(B, NP, 128).transpose(0, 2, 1).astype(dt))

    fx_dev = slot2dev(fx_slot, _bf16)
    b_dev = [slot2dev(((off_slot >> k) & 1), np.uint8) for k in (3, 2, 1, 0)]

    # dma_gather index tensor: [128, 8*NP] int16; for pass p, index i (lane) at
    # [i % 16, 8*p + i // 16], replicated over partition groups of 16.
    idx16 = np.zeros((B, 16, 8 * NP), np.int16)
    lanes = np.arange(128)
    for p in range(NP):
        v = idx[:, p * 128:(p + 1) * 128]            # (B,128)
        idx16[:, lanes % 16, p * 8 + lanes // 16] = v.astype(np.int16)
    idx16 = np.tile(idx16, (1, 8, 1))                # [B,128,8NP]

    # --- prescaled gathered operands (bf16) ---
    i1hw = np.ascontiguousarray(input1.transpose(0, 2, 3, 1).reshape(B, HW, C))
    w0 = (i1hw * (1.0 - fyr)[:, :, None]).astype(_bf16)
    w1 = (i1hw * fyr[:, :, None]).astype(_bf16)

    # --- padded resident i2 (bf16), layout [B, 128(c1), 2(c2), Hp, Wp] ---
    y0v = np.array([k[0] for k in chunk_list]); xbv = np.array([k[1] for k in chunk_list])
    row_lo = int(y0v.min()) - 4; row_hi = int(y0v.max()) + 5
    col_lo = int(xbv.min()) * XBW - 4; col_hi = int(xbv.max()) * XBW + (XBW - 1) + 5
    Hp = row_hi - row_lo + 1; Wp = col_hi - col_lo + 1
    ybase = -row_lo; xbase = -col_lo
    i2p = np.zeros((B, 128, 2, Hp, Wp), _bf16)
    i2p[:, :, :, ybase:ybase + H, xbase:xbase + W] = \
        input2.reshape(B, 2, 128, H, W).transpose(0, 2, 1, 3, 4).astype(_bf16)

    chunks = [(int(k[0]), int(k[1])) for k in chunk_list]
    meta = dict(NP=NP, Hp=Hp, Wp=Wp, ybase=ybase, xbase=xbase, chunks=chunks)
    tensors = dict(w0=w0, w1=w1,
                   i2p=i2p.reshape(B, 128, 2 * Hp * Wp),
                   idx16=idx16, fxp=fx_dev,
                   b8=b_dev[0], b4=b_dev[1], b2=b_dev[2], b1=b_dev[3])
    return meta, tensors, pixmap


def _build_module(meta):
    NP = meta["NP"]; Hp = meta["Hp"]; Wp = meta["Wp"]
    ybase = meta["ybase"]; xbase = meta["xbase"]; chunks = meta["chunks"]
    bf = mybir.dt.bfloat16
    f32 = mybir.dt.float32

    nc = bacc.Bacc("TRN2", target_bir_lowering=False, debug=False,
                   enable_asserts=False, num_devices=B)
    w0_d = nc.dram_tensor("w0", [HW, C], bf, kind="ExternalInput").ap()
    w1_d = nc.dram_tensor("w1", [HW, C], bf, kind="ExternalInput").ap()
    i2_d = nc.dram_tensor("i2p", [128, 2 * Hp * Wp], bf, kind="ExternalInput").ap()
    idx_d = nc.dram_tensor("idx16", [128, 8 * NP], mybir.dt.int16, kind="ExternalInput").ap()
    fxp_d = nc.dram_tensor("fxp", [128, NP], bf, kind="ExternalInput").ap()
    bit_d = [nc.dram_tensor(n, [128, NP], mybir.dt.uint8, kind="ExternalInput").ap()
             for n in ("b8", "b4", "b2", "b1")]
    out_d = nc.dram_tensor("corr", [128, NP, 81], f32, kind="ExternalOutput").ap()

    with tile.TileContext(nc, trace_sim=False) as tc:
        with tc.tile_pool(name="resident", bufs=1) as res_pool, \
             tc.tile_pool(name="ga", bufs=6) as ga_pool, \
             tc.tile_pool(name="psum", bufs=8, space="PSUM") as psum_pool, \
             tc.tile_pool(name="post", bufs=3) as post_pool, \
             tc.tile_pool(name="ost", bufs=3) as ost_pool:

            i2s = res_pool.tile([128, 2, Hp, Wp], bf, tag="i2s")
            nc.sync.dma_start(i2s[:], i2_d.rearrange("p (k h w) -> p k h w", k=2, h=Hp, w=Wp))
            idxs = res_pool.tile([128, 8 * NP], mybir.dt.int16, tag="idxs")
            nc.sync.dma_start(idxs[:], idx_d[:])
            fxp = res_pool.tile([128, NP], bf, tag="fxp")
            nc.sync.dma_start(fxp[:], fxp_d[:])
            bits = []
            for i, bd in enumerate(bit_d):
                t = res_pool.tile([128, NP], mybir.dt.uint8, tag=f"bit{i}")
                nc.sync.dma_start(t[:], bd[:])
                bits.append(t)

            for p0 in range(0, NP, NB):
                nb = min(NB, NP - p0)
                ebuf = post_pool.tile([128, NB, 9, WIN], bf, tag="ebuf")
                for i in range(nb):
                    p = p0 + i
                    ga0 = ga_pool.tile([128, 2, 128], bf, tag="ga0")
                    nc.gpsimd.dma_gather(ga0[:], w0_d[:], idxs[:, p * 8:(p + 1) * 8],
                                         128, 128, C, transpose=True)
                    ga1 = ga_pool.tile([128, 2, 128], bf, tag="ga1")
                    nc.gpsimd.dma_gather(ga1[:], w1_d[:], idxs[:, p * 8:(p + 1) * 8],
                                         128, 128, C, transpose=True)
                    ps = psum_pool.tile([128, 9, WIN], f32, tag="ps")
                    for g in range(4):
                        y0v, xbv = chunks[4 * p + g]
                        r0 = ybase + y0v - 4
                        c0 = xbase + XBW * xbv - 4
                        pslice = ps[g * 32:(g + 1) * 32, :, :]
                        for k in range(2):
                            nc.tensor.matmul(
                                pslice, ga0[:, k, g * 32:(g + 1) * 32],
                                i2s[:, k, r0:r0 + 9, c0:c0 + WIN],
                                start=(k == 0), stop=False,
                                tile_position=(0, g * 32))
                        for k in range(2):
                            nc.tensor.matmul(
                                pslice, ga1[:, k, g * 32:(g + 1) * 32],
                                i2s[:, k, r0 + 1:r0 + 10, c0:c0 + WIN],
                                start=False, stop=(k == 1),
                                tile_position=(0, g * 32))
                    # evacuate + cast
                    nc.scalar.copy(ebuf[:, i], ps[:])

                # barrel-shift extract: off = b8*8 + b4*4 + b2*2 + b1
                def bsel(bt):
                    return bt[:, p0:p0 + nb].unsqueeze(2).unsqueeze(3)

                widths = (17, 13, 11, 10)
                shifts = (8, 4, 2, 1)
                cur = ebuf[:, :nb]
                for si in range(4):
                    w = widths[si]; sh = shifts[si]
                    t = post_pool.tile([128, NB, 9, w], bf, tag=f"t{w}")
                    nc.scalar.copy(t[:, :nb], cur[:, :, :, 0:w])
                    nc.vector.copy_predicated(
                        t[:, :nb],
                        bsel(bits[si]).broadcast_to([128, nb, 9, w]),
                        cur[:, :, :, sh:sh + w])
                    cur = t[:, :nb]
                # x-blend: corr[a,dx] = t10[a,dx] + fx*(t10[a,dx+1]-t10[a,dx])
                d = post_pool.tile([128, NB, 9, 9], bf, tag="xd")
                nc.vector.tensor_sub(d[:, :nb], cur[:, :, :, 1:10], cur[:, :, :, 0:9])
                m = post_pool.tile([128, NB, 9, 9], bf, tag="xm")
                nc.vector.tensor_mul(
                    m[:, :nb], d[:, :nb],
                    fxp[:, p0:p0 + nb].unsqueeze(2).unsqueeze(3).broadcast_to([128, nb, 9, 9]))
                ost = ost_pool.tile([128, NB, 9, 9], f32, tag="ost")
                nc.vector.tensor_add(ost[:, :nb], m[:, :nb], cur[:, :, :, 0:9])
                nc.sync.dma_start(
                    out_d[:, p0:p0 + nb].rearrange("q n (a b) -> q n a b", a=9, b=9),
                    ost[:, :nb])

    nc.compile()
    return nc


def kernel(input1, input2, flow):
    input1 = np.asarray(input1); input2 = np.asarray(input2); flow = np.asarray(flow)
    assert input1.shape == (B, C, H, W)
    meta, tensors, pixmap = _host_prep(input1.astype(np.float32),
                                       input2.astype(np.float32),
                                       flow.astype(np.float32))
    nc = _build_module(meta)
    in_maps = [{k: np.ascontiguousarray(v[b]) for k, v in tensors.items()}
               for b in range(B)]
    res = bass_utils.run_bass_kernel_spmd(nc, in_maps, core_ids=list(range(B)))
    kernel.last_results = res
    kernel.last_nc = nc
    kernel.last_in_maps = in_maps

    NP = meta["NP"]
    out = np.zeros((B, 81, HW), np.float32)
    for b in range(B):
        corr = res.results[b]["corr"]            # [128, NP, 81]
        flat = corr.transpose(1, 0, 2).reshape(NP * 128, 81)
        pm = pixmap[b].reshape(NP * 128)
        sel = pm >= 0
        out[b, :, pm[sel]] = flat[sel]
    return np.ascontiguousarray(
        out.reshape(B, 9, 9, H, W))
